# revision 3
# baseline (speedup 1.0000x reference)
"""DeformMCALayer Trainium2 kernel v3: SBUF-source transposing gather,
single packed input tensor, fp16 output.

Per core (one image):
  1. offset conv (3x3, f32r matmuls, PSUM-accumulated) -> off_sb [18, 4096]
  2. PE-transpose offsets to pixel-partition; DVE index/bilinear-weight math
  3. idx 16-wrap via two-stage PE transposes
  4. w4 -> w4f [36, 4096] broadcast rows (PE transposes + ACT + row DMAs)
  5. x kept in SBUF as pair-token layout xsrc [128, 64, 256] bf16;
     SWDGE SBUF-source dma_gather(transpose=True, single_packet=False)
     fetches (pixel, pixel+1) x 256ch -> channel-on-partition tiles
  6. weight broadcast via selector matmuls into f32 PSUM, ACT-copied to
     bf16 SBUF; DVE all-bf16 weighted 4-corner sum -> patchT
  7. main matmul vs w2 (contraction 2304), f32 PSUM
  8. channel attention fused on ACT (accum_out); fp16 y output

All inputs packed into one `consts` f32 tensor (offsets in _OFF).
"""
import sys
sys.path.insert(0, "/opt/trn_rl_repo")
import numpy as np
import ml_dtypes

import concourse.bacc as bacc
import concourse.mybir as mybir
from concourse.tile import TileContext
from concourse.ap import AP

F32 = mybir.dt.float32
F32R = mybir.dt.float32r
F16 = mybir.dt.float16
BF16 = mybir.dt.bfloat16
I16 = mybir.dt.int16
OP = mybir.AluOpType
AF = mybir.ActivationFunctionType

H = W = 64
HW = 4096
K = 9
NQT = 4
QPIX = 1024
N_CORES = 8
MAGIC = float(3 * 2 ** 22)

# consts layout (f32 element offsets; bf16 segments use 2x element offsets)
_SIZES = dict(
    x=256 * HW,                 # f32
    offw=128 * 2 * K * 18,      # f32 (used as f32r)
    basey=128 * 32 * K,         # f32
    basex=128 * 32 * K,         # f32
    idf=128 * 128,              # f32
    w2=128 * 18 * 256 // 2,     # bf16 pairs packed in f32 slots
    bsel=36 * 36 * 128 // 2,    # bf16 pairs
)
_OFF = {}
_o = 0
for _k, _s in _SIZES.items():
    _OFF[_k] = _o
    _o += _s
NF32 = _o


def _mk(ap_or_handle, extra_offset, dims):
    if isinstance(ap_or_handle, AP):
        t, off = ap_or_handle.tensor, ap_or_handle.offset
    else:
        a = ap_or_handle.ap()
        t, off = a.tensor, a.offset
    return AP(t, off + extra_offset, [list(d) for d in dims])


def build_program(repeat=1, stage=4, nq=2):
    nc = bacc.Bacc("TRN2", target_bir_lowering=False, debug=False, num_devices=1,
                   num_swdge_queues=nq)

    c_d = nc.dram_tensor("consts", [1, NF32], F32, kind="ExternalInput")
    y_d = nc.dram_tensor("y", [256, HW], F16, kind="ExternalOutput")

    ct = c_d.ap().tensor

    def cap(name, dims, dtype=F32):
        # dims in f32 units; bitcast scales offset/strides/last-count
        ap = AP(ct, _OFF[name], [list(d) for d in dims])
        return ap if dtype == F32 else ap.bitcast(dtype)

    with TileContext(nc) as tc:
        for _rep in range(repeat):
            with tc.tile_pool(name="const", bufs=1) as cpool:
                w2_sb = cpool.tile([128, 18, 256], BF16)
                nc.sync.dma_start(
                    w2_sb[:], cap("w2", [[2304, 128], [1, 2304]], BF16))
                offw_sb = cpool.tile([128, 2, K, 18], F32R)
                nc.sync.dma_start(
                    offw_sb[:],
                    cap("offw", [[2 * K * 18, 128], [1, 2 * K * 18]], F32R))
                basey_sb = cpool.tile([128, 32, K], F32)
                nc.sync.dma_start(
                    basey_sb[:], cap("basey", [[32 * K, 128], [1, 32 * K]]))
                basex_sb = cpool.tile([128, 32, K], F32)
                nc.sync.dma_start(
                    basex_sb[:], cap("basex", [[32 * K, 128], [1, 32 * K]]))
                idf_sb = cpool.tile([128, 128], F32)
                nc.sync.dma_start(idf_sb[:], cap("idf", [[128, 128], [1, 128]]))
                sel = cpool.tile([36, 36, 128], BF16)
                nc.sync.dma_start(
                    sel[:], cap("bsel", [[2304, 36], [1, 2304]], BF16))

                xsrc = cpool.tile([128, 64, 256], BF16)
                w4f = cpool.tile([36, HW], BF16)
                idxw = cpool.tile([128, K, 2, NQT, 64], I16)
                offT = cpool.tile([128, 32, 18], F32)
                w4 = cpool.tile([128, 32, K, 4], F32)

                # ---------------- boot: conv + transposes ----------------
                with tc.tile_pool(name="boot", bufs=1) as bpool, \
                     tc.tile_pool(name="psconv", bufs=2, space="PSUM") as psconv, \
                     tc.tile_pool(name="pstp", bufs=2, space="PSUM") as pstp:
                    x_pad = bpool.tile([128, 2, 66, 66], F32R)
                    nc.vector.memset(x_pad[:, :, 0, :].bitcast(F32), 0.0)
                    nc.vector.memset(x_pad[:, :, 65, :].bitcast(F32), 0.0)
                    nc.vector.memset(x_pad[:, :, 1:65, 0:1].bitcast(F32), 0.0)
                    nc.vector.memset(x_pad[:, :, 1:65, 65:66].bitcast(F32), 0.0)
                    for cb in range(2):
                        src = AP(ct, _OFF["x"] + cb * 128 * HW,
                                 [[HW, 128], [64, 64], [1, 64]])
                        nc.gpsimd.dma_start(x_pad[:, cb, 1:65, 1:65], src)

                    off_sb = bpool.tile([18, HW], F32)
                    for chk in range(8):
                        ps_conv = psconv.tile([18, 512], F32, tag="conv")
                        r0 = chk * 8
                        idx = 0
                        for cb in range(2):
                            for k in range(K):
                                ky, kx = k // 3, k % 3
                                rhs = x_pad[:, cb, r0 + ky: r0 + ky + 8, kx: kx + 64]
                                nc.tensor.matmul(
                                    ps_conv[:], offw_sb[:, cb, k, :], rhs,
                                    start=(idx == 0), stop=(idx == 17))
                                idx += 1
                        nc.scalar.copy(off_sb[:, chk * 512:(chk + 1) * 512], ps_conv[:])

                    for b in range(32):
                        ps_t = pstp.tile([128, 18], F32, tag="tp18")
                        nc.tensor.transpose(ps_t[:], off_sb[:, b * 128:(b + 1) * 128],
                                            idf_sb[0:18, 0:18])
                        nc.scalar.copy(offT[:, b, :], ps_t[:])

                    x_g = bpool.tile([128, 2, 64 + HW + 64], F32)
                    nc.vector.memset(x_g[:, :, 0:64], 0.0)
                    nc.vector.memset(x_g[:, :, 64 + HW:], 0.0)
                    for cb in range(2):
                        nc.vector.tensor_copy(
                            x_g[:, cb, 64:64 + HW],
                            x_pad[:, cb, 1:65, 1:65].bitcast(F32))

                    for y in range(64):
                        for cb in range(2):
                            pt = pstp.tile([128, 128], F32, tag="tpx")
                            inB = x_g[:, cb, y * 64 + 1: y * 64 + 129]
                            nc.tensor.transpose(pt[:], inB, idf_sb[:])
                            inA = x_g[:, cb, 64 + y * 64: 64 + y * 64 + 64]
                            nc.tensor.transpose(pt[0:64, :], inA, idf_sb[:])
                            nc.scalar.copy(xsrc[:, y, cb * 128:(cb + 1) * 128],
                                           pt[:])

                # ---------------- index & weight math ----------------
                with tc.tile_pool(name="idx", bufs=1) as ipool, \
                     tc.tile_pool(name="psw", bufs=2, space="PSUM") as psw:
                    def it(name):
                        return ipool.tile([128, 32, K], F32, tag=name, name=name)

                    sy = it("sy"); sx = it("sx")
                    nc.vector.tensor_add(sy[:], basey_sb[:], offT[:, :, 0:K])
                    nc.vector.tensor_add(sx[:], basex_sb[:], offT[:, :, K:18])

                    def floor_(s_t, name):
                        t = it(name + "_t"); c = it(name + "_c")
                        f = it(name + "_f"); l = it(name + "_l")
                        nc.vector.tensor_scalar_add(t[:], s_t[:], MAGIC)
                        nc.vector.tensor_scalar_sub(t[:], t[:], MAGIC)
                        nc.vector.tensor_tensor(c[:], t[:], s_t[:], OP.is_gt)
                        nc.vector.tensor_sub(f[:], t[:], c[:])
                        nc.vector.tensor_sub(l[:], s_t[:], f[:])
                        return f, l

                    y0, ly = floor_(sy, "y")
                    x0, lx = floor_(sx, "x")

                    yc0 = it("yc0"); yc1 = it("yc1"); xc0 = it("xc0"); y1 = it("y1")
                    nc.vector.tensor_scalar(yc0[:], y0[:], 0.0, 63.0, OP.max, OP.min)
                    nc.vector.tensor_scalar_add(y1[:], y0[:], 1.0)
                    nc.vector.tensor_scalar(yc1[:], y1[:], 0.0, 63.0, OP.max, OP.min)
                    nc.vector.tensor_scalar(xc0[:], x0[:], 0.0, 63.0, OP.max, OP.min)

                    def vmask(src_t, lo, hi, name):
                        a = it(name + "_a"); b = it(name + "_b"); v = it(name + "_v")
                        nc.vector.tensor_scalar(a[:], src_t[:], float(lo), None, OP.is_ge)
                        nc.vector.tensor_scalar(b[:], src_t[:], float(hi), None, OP.is_le)
                        nc.vector.tensor_mul(v[:], a[:], b[:])
                        return v

                    vy0 = vmask(y0, 0, 63, "vy0")
                    vy1 = vmask(y1, 0, 63, "vy1")
                    vx0 = vmask(x0, 0, 63, "vx0")
                    cx62 = vmask(x0, 0, 62, "cx62")
                    ex = it("ex")
                    nc.vector.tensor_scalar(ex[:], x0[:], -1.0, None, OP.is_equal)

                    oly = it("oly"); olx = it("olx")
                    nc.vector.tensor_scalar(oly[:], ly[:], -1.0, 1.0, OP.mult, OP.add)
                    nc.vector.tensor_scalar(olx[:], lx[:], -1.0, 1.0, OP.mult, OP.add)

                    wy0 = it("wy0"); wy1 = it("wy1"); wx0 = it("wx0"); wx1 = it("wx1")
                    nc.vector.tensor_mul(wy0[:], oly[:], vy0[:])
                    nc.vector.tensor_mul(wy1[:], ly[:], vy1[:])
                    t1 = it("t1"); t2 = it("t2")
                    nc.vector.tensor_mul(t1[:], olx[:], vx0[:])
                    nc.vector.tensor_mul(t2[:], lx[:], ex[:])
                    nc.vector.tensor_add(wx0[:], t1[:], t2[:])
                    nc.vector.tensor_mul(wx1[:], lx[:], cx62[:])

                    for s, (a_t, b_t) in enumerate([(wy0, wx0), (wy0, wx1),
                                                    (wy1, wx0), (wy1, wx1)]):
                        nc.vector.tensor_tensor(w4[:, :, :, s], a_t[:], b_t[:], OP.mult)

                    ida = it("ida"); idb_t = it("idb"); m1 = it("m1"); m2 = it("m2")
                    nc.vector.tensor_scalar_mul(m1[:], yc0[:], 64.0)
                    nc.vector.tensor_add(ida[:], m1[:], xc0[:])
                    nc.vector.tensor_scalar_mul(m2[:], yc1[:], 64.0)
                    nc.vector.tensor_add(idb_t[:], m2[:], xc0[:])

                    idxf = ipool.tile([128, K, 2, 32], F32, tag="idxf")
                    for ab, src_t in enumerate([ida, idb_t]):
                        src_ap = _mk(src_t[:], 0, [list(src_t[:].ap[0]), [1, K], [K, 32]])
                        dst_ap = _mk(idxf[:], ab * 32, [list(idxf[:].ap[0]), [64, K], [1, 32]])
                        nc.vector.tensor_copy(dst_ap, src_ap)

                    idxw16 = ipool.tile([16, K, 2, NQT, 64], I16, tag="idxw16")
                    T1_sb = ipool.tile([128, 5, 128], F32, tag="T1")
                    nc.vector.memset(T1_sb[:], 0.0)
                    widths = [128, 128, 128, 128, 64]
                    for ch in range(5):
                        wd = widths[ch]
                        ps = psw.tile([128, 128], F32, tag="tpw")
                        in_ap = _mk(idxf[:], ch * 128, [list(idxf[:].ap[0]), [1, wd]])
                        nc.tensor.transpose(ps[0:wd, :], in_ap, idf_sb[:])
                        nc.scalar.copy(T1_sb[0:wd, ch, :], ps[0:wd, :])
                    nc.vector.memset(idxw16[:], 0)
                    for q in range(8):
                        for ch in range(5):
                            wd = widths[ch]
                            ps2 = psw.tile([16, 128], F32, tag="tpw2")
                            in2 = T1_sb[:, ch, q * 16: q * 16 + 16]
                            nc.tensor.transpose(ps2[:], in2, idf_sb[:])
                            base = idxw16[:].offset + (ch * 2) * 512 + q
                            pa = list(idxw16[:].ap[0])
                            pa[1] = 16
                            if wd == 128:
                                dims = [pa, [512, 2], [256, 2], [64, 4], [8, 8]]
                            else:
                                dims = [pa, [256, 2], [64, 4], [8, 8]]
                            dst_ap = AP(idxw16[:].tensor, base, dims)
                            nc.vector.tensor_copy(dst_ap, ps2[:, 0:wd])
                    for cgrp in range(8):
                        nc.sync.dma_start(idxw[cgrp * 16:(cgrp + 1) * 16], idxw16[:])

                    w4T = ipool.tile([32, 36, 128], BF16, tag="w4T")
                    for k in range(K):
                        for c in range(4):
                            j = k * 4 + c
                            psj = psw.tile([32, 128], F32, tag="tpw4")
                            in_ap = _mk(w4[:], k * 4 + c,
                                        [list(w4[:].ap[0]), [K * 4, 32]])
                            nc.tensor.transpose(psj[:], in_ap, idf_sb[:])
                            nc.scalar.copy(w4T[:, j, :], psj[:])
                    for j in range(36):
                        nc.sync.dma_start(w4f[j:j + 1, :], w4T[:, j, :])

                # ---------------- main: gather, weight, matmul ----------
                with tc.tile_pool(name="main", bufs=2) as mpool, \
                     tc.tile_pool(name="ybuf", bufs=1) as ypool, \
                     tc.tile_pool(name="pswb", bufs=2, space="PSUM") as pswb, \
                     tc.tile_pool(name="psmm", bufs=3, space="PSUM") as psmm:
                    y_sb = ypool.tile([128, 2, HW], F16)
                    s1p = ypool.tile([128, 2, 8], F32, name="s1p")
                    s2p = ypool.tile([128, 2, 8], F32, name="s2p")

                    for qt in range(NQT):
                        patchT = mpool.tile([128, 18, QPIX], BF16, tag="patchT",
                                            bufs=1)
                        for k in range(K):
                            gtop = mpool.tile([128, 4, QPIX], BF16, tag="gt", bufs=3)
                            gbot = mpool.tile([128, 4, QPIX], BF16, tag="gb", bufs=3)
                            if stage >= 1:
                                nc.gpsimd.dma_gather(
                                    gtop[:], xsrc[:], idxw[:, k, 0, qt, :],
                                    QPIX, QPIX, 512, transpose=True,
                                    single_packet=False, queue_num=0,
                                    sbuf_tokens_per_rank=64,
                                    sbuf_free_dim_per_rank=512,
                                    sbuf_free_dim_pad_per_rank=0,
                                    sbuf_byte_offset=0)
                                nc.gpsimd.dma_gather(
                                    gbot[:], xsrc[:], idxw[:, k, 1, qt, :],
                                    QPIX, QPIX, 512, transpose=True,
                                    single_packet=False, queue_num=0,
                                    sbuf_tokens_per_rank=64,
                                    sbuf_free_dim_per_rank=512,
                                    sbuf_free_dim_pad_per_rank=0,
                                    sbuf_byte_offset=0)
                            else:
                                nc.vector.memset(gtop[:].bitcast(I16), 0)
                                nc.vector.memset(gbot[:].bitcast(I16), 0)

                            if stage >= 2:
                                wsT = mpool.tile([128, 2, QPIX], BF16, tag="wsT")
                                wsB = mpool.tile([128, 2, QPIX], BF16, tag="wsB")
                                for h in range(2):
                                    csl = slice(qt * QPIX + h * 512,
                                                qt * QPIX + (h + 1) * 512)
                                    wpT = pswb.tile([128, 2, 512], F32, tag="wb")
                                    for c in range(2):
                                        nc.tensor.matmul(
                                            wpT[:, c, :], sel[:, k * 4 + c, :],
                                            w4f[0:36, csl],
                                            start=True, stop=True)
                                    wpB = pswb.tile([128, 2, 512], F32, tag="wb")
                                    for c in range(2):
                                        nc.tensor.matmul(
                                            wpB[:, c, :], sel[:, k * 4 + 2 + c, :],
                                            w4f[0:36, csl],
                                            start=True, stop=True)
                                    hs = slice(h * 512, (h + 1) * 512)
                                    nc.scalar.copy(wsT[:, :, hs], wpT[:])
                                    nc.scalar.copy(wsB[:, :, hs], wpB[:])
                                for cb in range(2):
                                    t0 = mpool.tile([128, QPIX], BF16, tag="t0", bufs=1)
                                    t1 = mpool.tile([128, QPIX], BF16, tag="t1", bufs=1)
                                    nc.vector.tensor_tensor(
                                        t0[:], gtop[:, cb, :], wsT[:, 0, :], OP.mult)
                                    nc.vector.tensor_tensor(
                                        t1[:], gtop[:, 2 + cb, :], wsT[:, 1, :], OP.mult)
                                    nc.vector.tensor_add(t0[:], t0[:], t1[:])
                                    t2 = mpool.tile([128, QPIX], BF16, tag="t2", bufs=1)
                                    t3 = mpool.tile([128, QPIX], BF16, tag="t3", bufs=1)
                                    nc.vector.tensor_tensor(
                                        t2[:], gbot[:, cb, :], wsB[:, 0, :], OP.mult)
                                    nc.vector.tensor_tensor(
                                        t3[:], gbot[:, 2 + cb, :], wsB[:, 1, :], OP.mult)
                                    nc.vector.tensor_add(t2[:], t2[:], t3[:])
                                    nc.vector.tensor_add(
                                        patchT[:, k * 2 + cb, :], t0[:], t2[:])

                        if stage >= 4:
                            for chunk in range(2):
                                for oh in range(2):
                                    psd = psmm.tile([128, 512], F32, tag="mm")
                                    for kc in range(18):
                                        nc.tensor.matmul(
                                            psd[:], w2_sb[:, kc, oh * 128:(oh + 1) * 128],
                                            patchT[:, kc, chunk * 512:(chunk + 1) * 512],
                                            start=(kc == 0), stop=(kc == 17))
                                    cidx = qt * 2 + chunk
                                    nc.scalar.activation(
                                        y_sb[:, oh, qt * 1024 + chunk * 512:
                                             qt * 1024 + (chunk + 1) * 512],
                                        psd[:], AF.Copy, accum_out=s1p[:, oh, cidx:cidx + 1])
                                    sqscr = mpool.tile([128, 512], BF16, tag="sq", bufs=1)
                                    nc.scalar.activation(
                                        sqscr[:], psd[:], AF.Square,
                                        accum_out=s2p[:, oh, cidx:cidx + 1])

                    # ---------------- stats + scale ----------------
                    if stage < 4:
                        nc.vector.memset(y_sb[:, 0, 0:64].bitcast(I16), 0)
                        nc.sync.dma_start(_mk(y_d, 0, [[HW, 128], [1, 64]]),
                                          y_sb[:, 0, 0:64])
                        continue
                    s1 = ypool.tile([128, 2], F32)
                    s2 = ypool.tile([128, 2], F32)
                    nc.vector.reduce_sum(s1[:], s1p[:], axis=mybir.AxisListType.X)
                    nc.vector.reduce_sum(s2[:], s2p[:], axis=mybir.AxisListType.X)
                    mean = ypool.tile([128, 2], F32)
                    nc.vector.tensor_scalar_mul(mean[:], s1[:], 1.0 / HW)
                    ss = ypool.tile([128, 2], F32)
                    nc.vector.tensor_mul(ss[:], s1[:], s1[:])
                    va = ypool.tile([128, 2], F32)
                    vb = ypool.tile([128, 2], F32)
                    var = ypool.tile([128, 2], F32)
                    nc.vector.tensor_scalar_mul(va[:], s2[:], 1.0 / (HW - 1))
                    nc.vector.tensor_scalar_mul(vb[:], ss[:], 1.0 / (HW * (HW - 1.0)))
                    nc.vector.tensor_sub(var[:], va[:], vb[:])
                    nc.vector.tensor_scalar_max(var[:], var[:], 0.0)
                    std = ypool.tile([128, 2], F32)
                    nc.scalar.sqrt(std[:], var[:])
                    arg = ypool.tile([128, 2], F32)
                    nc.vector.tensor_add(arg[:], mean[:], std[:])
                    attn = ypool.tile([128, 2], F32)
                    nc.scalar.activation(attn[:], arg[:], AF.Sigmoid)
                    for oh in range(2):
                        nc.vector.tensor_scalar_mul(y_sb[:, oh, :], y_sb[:, oh, :],
                                                    attn[:, oh:oh + 1])
                        nc.sync.dma_start(
                            _mk(y_d, oh * 128 * HW, [[HW, 128], [1, HW]]),
                            y_sb[:, oh, :])

    nc.compile()
    return nc


def _prep_shared(offset_w, deform_w):
    """Shared (per-weights) tail of the consts tensor, f32 view."""
    perm = [2 * i for i in range(9)] + [2 * i + 1 for i in range(9)]
    wp = np.asarray(offset_w, np.float32)[perm]
    wp2 = wp.reshape(18, 2, 128, 9)
    offw = np.ascontiguousarray(wp2.transpose(2, 1, 3, 0)).astype(np.float32)

    wk = np.asarray(deform_w, np.float32).reshape(256, 256, 9)
    t = wk.reshape(256, 2, 128, 9).transpose(2, 3, 1, 0)
    w2 = np.ascontiguousarray(t.reshape(128, 18, 256)).astype(ml_dtypes.bfloat16)

    p = np.arange(128)
    blk = np.arange(32)
    kk = np.arange(9)
    i_pix = blk[None, :, None] * 2 + (p[:, None, None] // 64)
    j_pix = (p[:, None, None] % 64) + 0 * blk[None, :, None]
    basey = np.ascontiguousarray(np.broadcast_to(
        (i_pix + (kk // 3)[None, None, :] - 1), (128, 32, 9))).astype(np.float32)
    basex = np.ascontiguousarray(np.broadcast_to(
        (j_pix + (kk % 3)[None, None, :] - 1), (128, 32, 9))).astype(np.float32)

    idf = np.eye(128, dtype=np.float32)
    bsel = np.zeros((36, 36, 128), np.float32)
    for j in range(36):
        bsel[j, j, :] = 1.0
    bsel = bsel.astype(ml_dtypes.bfloat16)

    tail = np.empty(NF32 - _SIZES["x"], np.float32)

    def put(name, arr):
        off = _OFF[name] - _SIZES["x"]
        if arr.dtype == ml_dtypes.bfloat16:
            fl = arr.reshape(-1).view(np.uint16).reshape(-1, 2)
            packed = fl[:, 0].astype(np.uint32) | (fl[:, 1].astype(np.uint32) << 16)
            tail[off:off + _SIZES[name]] = packed.view(np.float32)
        else:
            tail[off:off + _SIZES[name]] = arr.reshape(-1)

    put("offw", offw)
    put("basey", basey)
    put("basex", basex)
    put("idf", idf)
    put("w2", w2)
    put("bsel", bsel)
    return tail


def _pack(x_b, tail):
    c = np.empty((1, NF32), np.float32)
    c[0, :_SIZES["x"]] = x_b.reshape(-1)
    c[0, _SIZES["x"]:] = tail
    return c


_CACHE = {}


def kernel(x, offset_w, deform_w):
    x = np.asarray(x, np.float32)
    B = x.shape[0]
    assert x.shape == (8, 256, 64, 64)

    if "nc" not in _CACHE:
        _CACHE["nc"] = build_program()
    nc = _CACHE["nc"]

    tail = _prep_shared(offset_w, deform_w)
    in_maps = [{"consts": _pack(x[b], tail)} for b in range(B)]

    from concourse.bass_utils import run_bass_kernel_spmd
    res = run_bass_kernel_spmd(nc, in_maps, core_ids=list(range(N_CORES)))
    out = np.stack([res.results[b]["y"].astype(np.float32).reshape(256, 64, 64)
                    for b in range(B)])
    return out

def _make_in_maps(x, offset_w, deform_w):
    tail = _prep_shared(offset_w, deform_w)
    return [{"consts": _pack(np.ascontiguousarray(
        np.asarray(x, np.float32)[b].reshape(256, HW)), tail)}
        for b in range(8)]
